# revision 66
# baseline (speedup 1.0000x reference)
"""FlowMamba Trainium2 kernel: 8-core SPMD, dm-sharded SBUF-resident SSM state.

Contract: kernel(**inputs) takes the full unsharded inputs (as in
reference.setup_inputs()) and returns the full output (B, pred_len, C, H, W)
float32.  Self-contained: hardcodes shapes/sharding.
"""

import os
import sys

for _p in ("/opt/trn_rl_repo", "/root/.axon_site/_ro/trn_rl_repo"):
    if os.path.isdir(_p) and _p not in sys.path:
        sys.path.insert(0, _p)

import numpy as np

import concourse.bass as bass
import concourse.mybir as mybir
import bass_rust as _bass_rust
from concourse.tile import TileContext
from concourse.bass_utils import run_bass_kernel_spmd

F32 = mybir.dt.float32
F16 = mybir.dt.float16
AF = mybir.ActivationFunctionType
OP = mybir.AluOpType

NCORES = 8
H = 32
NP = 1024          # pixels
PW = 34            # padded width
NPAD = PW * PW     # 1156
PPAD = NPAD + 70   # padded-with-slack for the 9-shift DMA trick
DM = 64
DS = 16
DSH = 8            # dm per core
NV = 25
V_LIST = [(x, y) for x in range(-2, 3) for y in range(-2, 3)]  # (vx, vy)

# ---------------------------------------------------------------------------
# Patch: this walrus build rejects >1 sync-wait on CTRL (Drain) instructions.
# Split the Tile kernel-tail drain's waits across multiple drain instructions.
_PATCHED = False


def _patch_tile_drain():
    global _PATCHED
    if _PATCHED:
        return
    _PATCHED = True

    def _patched(self, tick_clock, wait_clock):
        from concourse.tile import ScopedClock

        nc = self.nc
        drain_inst = nc.sync.drain()
        wait_clock.add_sem_waits(
            drain_inst.ins, ScopedClock({None: tick_clock.global_clock})
        )
        si = drain_inst.ins.sync_info
        waits = list(si.on_wait) if si and si.on_wait else []
        if len(waits) > 1:
            si.on_wait = waits[:1]
            for i in range(1, len(waits)):
                extra = nc.sync.drain()
                extra.ins.sync_info = mybir.SyncInfo(
                    on_wait=[waits[i]], on_update=[]
                )
        nc.all_engine_barrier()
        assert self.sems is not None
        popped = nc._tile_sem_poison_stack.pop()
        assert popped is self._sem_poison
        nc.clear_and_free_semaphores(list(self.sems.allocated().values()))
        nc.all_engine_barrier()

    TileContext._drain_and_barrier = _patched



def _detach_last(nc, inst):
    for f in nc.m.functions:
        for bb in f.blocks:
            il = bb.instructions
            if il and il[-1].name == inst.name:
                bb.instructions = il[:-1]
                return inst
    raise RuntimeError("carrier not found in any block")


def _mk_carrier(nc, engine):
    import concourse.mybir as _mb

    if engine in (_mb.EngineType.DVE, _mb.EngineType.Pool):
        bi = nc.engines[engine].isa(
            nc.isa.Opcode.NEURON_ISA_TPB_OPCODE_ENGINE_NOP, {}
        )
    else:
        bi = nc.engines[engine].drain()
    return _detach_last(nc, bi.ins)


def _split_waits(nc):
    """This walrus build supports only one sync-wait command per instruction.
    Move one excess wait onto the immediately-preceding same-engine
    instruction when free (equivalent), and carry the rest on inserted
    same-engine nop/drain instructions."""
    for f in nc.m.functions:
        for bb in f.blocks:
            il = list(bb.instructions)
            out = []
            prev_by_engine = {}
            changed = False
            for inst in il:
                si = inst.sync_info
                waits = list(si.on_wait) if si and si.on_wait else []
                if len(waits) > 1:
                    changed = True
                    prev = prev_by_engine.get(inst.engine)
                    if prev is not None:
                        psi = prev.sync_info
                        # only absorb into a prev with no waits AND no updates
                        # (waiting before an update could deadlock the chain)
                        if psi is None:
                            prev.sync_info = mybir.SyncInfo(
                                on_wait=[waits.pop(0)], on_update=[]
                            )
                        elif not psi.on_wait and not psi.on_update:
                            psi.on_wait = [waits.pop(0)]
                    while len(waits) > 1:
                        w = waits.pop(0)
                        car = _mk_carrier(nc, inst.engine)
                        car.sync_info = mybir.SyncInfo(on_wait=[w], on_update=[])
                        out.append(car)
                        prev_by_engine[inst.engine] = car
                    si.on_wait = waits
                out.append(inst)
                prev_by_engine[inst.engine] = inst
            if changed:
                bb.instructions = out


# ---------------------------------------------------------------------------
# roll rectangles: out[h, w] = in[(h+vy) % 32, (w+vx) % 32]
def _segs(shift):
    s = shift % 32
    if s == 0:
        return [(0, 0, 32)]
    return [(0, s, 32 - s), (32 - s, 0, s)]


def _rects(vy, vx):
    out = []
    for oh, ih, hl in _segs(vy):
        for ow, iw, wl in _segs(vx):
            out.append((oh, ih, hl, ow, iw, wl))
    return out


def _vel_cost(v, eng, mode):
    """v1 cost-model ns for velocity v's per-cell ops on engine D/G.
    DVE TensorTensor fp16 SBUF: 0.521 ns/elem + ~60 ns busy per op (init).
    Pool TensorTensor: 0.833 ns/elem flat, no per-op busy adder.
    mode 'upd': rolled mult + add.  mode 'y': rolled mult only."""
    vx, vy = V_LIST[v]
    r = len(_rects(vy, vx))
    if eng == "D":
        mul = NP * 0.521 + 60.4 * r
        add = NP * 0.521 + 60.4
    else:
        mul = NP * 0.833
        add = NP * 0.833
    return mul + add if mode == "upd" else mul


def _assign_engines(mode):
    """LPT greedy minimizing makespan across DVE/Pool. Returns set on Pool.
    For the y split, DVE starts preloaded with the 25 maxes (+combine) it
    must run regardless of which engine does the rolled mult."""
    order = sorted(range(NV), key=lambda v: -_vel_cost(v, "D", mode))
    # y: DVE preloaded with the 25 maxes; Pool preloaded with the
    # pre-launch update slice that shares its queue before the collective.
    # upd-dec: Pool preloaded with the 18.3us collective minus the chain
    # slack it overlaps (net ~5us of effective queue pressure).
    if mode == "y":
        loads = {"D": 25 * 194.0 + 600.0, "G": 3400.0}
    elif mode == "upd-dec":
        loads = {"D": 0.0, "G": 5000.0}
    else:
        loads = {"D": 0.0, "G": 0.0}
    mode = "upd" if mode == "upd-dec" else mode
    gp = set()
    for v in order:
        if loads["D"] + _vel_cost(v, "D", mode) <= loads["G"] + _vel_cost(v, "G", mode):
            loads["D"] += _vel_cost(v, "D", mode)
        else:
            loads["G"] += _vel_cost(v, "G", mode)
            gp.add(v)
    return gp


GP_VS_UPD = _assign_engines("upd")          # enc cells (no y-phase)
GP_VS_UPD_DEC = _assign_engines("upd-dec")  # dec cells: Pool also carries the collective
GP_VS_Y = _assign_engines("y")


# ---------------------------------------------------------------------------
def build_program(t_in, plen):
    _patch_tile_drain()
    nc = bass.Bass()
    n_pro = t_in + plen - 1
    n_cells = t_in + plen

    # ---- dram inputs (shared across cores unless noted per-core) ----
    def din(name, shape, dt):
        return nc.dram_tensor(name, shape, dt, kind="ExternalInput")

    d_ipad = din("ipad", [t_in, PPAD], F16)          # padded input frames
    d_enc1 = din("enc1w", [9, DM], F16)
    d_enc2p = din("enc2p", [128, 3, DM], F16)
    d_enc2s = din("enc2s", [64, 3, DM], F16)
    d_combop = din("combop", [128, 3, 80], F16)      # per-core (wd slice)
    d_combos = din("combos", [64, 3, 80], F16)       # per-core
    d_dec1p = din("dec1p", [128, 3, DM], F16)
    d_dec1s = din("dec1s", [64, 3, DM], F16)
    d_dec2p = din("dec2p", [128, 3, DM], F16)
    d_dec2s = din("dec2s", [64, 3, DM], F16)
    d_dec3p = din("dec3p", [128, 3, 1], F16)
    d_dec3s = din("dec3s", [64, 3, 1], F16)
    d_Ed = din("Ed", [8, 128], F16)
    d_Gu = din("Gu", [64, 128], F16)                 # per-core (u shard select)
    d_Fs = din("Fs", [16, 128], F16)
    d_Msel = din("Msel", [128, 1024], F16)
    d_biasp = din("biasp", [128, 9], F32)            # per-core packed scalars:
    # cols: 0 App, 1 invApp, 2 dbias(rows 0..8), 3 eb1, 4 eb2, 5 db1,
    #       6 db2, 7 dsk, 8 db3(row 0)

    d_out = nc.dram_tensor("preds", [plen, NP], F16, kind="ExternalOutput")

    with TileContext(nc) as tc:
        import contextlib

        ctx = contextlib.ExitStack()
        with ctx:
            wpool = ctx.enter_context(tc.tile_pool(name="wgt", bufs=1))
            state = ctx.enter_context(tc.tile_pool(name="state", bufs=1))
            probuf = ctx.enter_context(tc.tile_pool(name="pro", bufs=2))
            bpool = ctx.enter_context(tc.tile_pool(name="bpool", bufs=2))
            small = ctx.enter_context(tc.tile_pool(name="small", bufs=1))
            pads2 = ctx.enter_context(tc.tile_pool(name="pads2", bufs=2))
            pads1 = ctx.enter_context(tc.tile_pool(name="pads1", bufs=2))
            wbuf = ctx.enter_context(tc.tile_pool(name="wbuf", bufs=6))
            ybuf = ctx.enter_context(tc.tile_pool(name="ybuf", bufs=3))
            cpool = ctx.enter_context(
                tc.tile_pool(name="cpsum", bufs=2, space="PSUM")
            )

            apool = cpool   # share one 2-slot ring for all big [128,1024] psums
            ypool = ctx.enter_context(
                tc.tile_pool(name="ypsum", bufs=2, space="PSUM")
            )
            gpool = ctx.enter_context(
                tc.tile_pool(name="gpsum", bufs=2, space="PSUM")
            )
            dram = ctx.enter_context(
                tc.tile_pool(name="dram", bufs=2, space="DRAM")
            )

            # ---- load weights/constants into SBUF ----
            def load(dt_tensor, shape, dtyp, tag):
                t = wpool.tile(shape, dtyp, tag=tag, name=tag)
                nc.sync.dma_start(t[:], dt_tensor[:])
                return t

            # ordered by first use: the SP DMA queue drains serially
            biasp = load(d_biasp, [128, 9], F32, "biasp")
            enc1w = load(d_enc1, [9, DM], F16, "enc1w")
            enc2p = load(d_enc2p, [128, 3, DM], F16, "enc2p")
            enc2s = load(d_enc2s, [64, 3, DM], F16, "enc2s")
            combop = load(d_combop, [128, 3, 80], F16, "combop")
            combos = load(d_combos, [64, 3, 80], F16, "combos")
            Ed = load(d_Ed, [8, 128], F16, "Ed")
            Gu = load(d_Gu, [64, 128], F16, "Gu")
            Fs = load(d_Fs, [16, 128], F16, "Fs")
            Msel = load(d_Msel, [128, 1024], F16, "Msel")
            App = biasp[0:128, 0:1]
            invApp = biasp[0:128, 1:2]
            dbias = biasp[0:8, 2:3]
            eb1 = biasp[0:64, 3:4]
            eb2 = biasp[0:64, 4:5]
            db1 = biasp[0:64, 5:6]
            db2 = biasp[0:64, 6:7]
            dsk = biasp[0:64, 7:8]
            db3 = biasp[0:1, 8:9]
            zpad = wpool.tile([64, NPAD], F16, tag="zpad", name="zpad")
            nc.gpsimd.memset(zpad[:], 0.0)
            # preload the (single) activation table while input DMAs run
            nc.scalar.activation(zpad[0:1, 0:1], zpad[0:1, 0:1], AF.Relu)

            # ---- persistent state slabs (fp16: DVE 2x mode) ----
            slabs = [
                state.tile([128, NP], F16, tag=f"slab{k}", name=f"slab{k}") for k in range(NV + 2)
            ]
            vtile = list(range(NV))      # vtile[v] = slab index holding state v
            spare = {"D": NV, "G": NV + 1}

            def v3(ap):  # [p, 1024] -> [p, 32, 32]
                return ap.rearrange("p (h w) -> p h w", h=H)

            def v34(ap):  # [p, 1156] -> [p, 34, 34]
                return ap.rearrange("p (r c) -> p r c", r=PW)

            def halos(pad, rows, eng=None):
                """Wrap halos of a [rows, 34, 34] pad in 2 packed copies:
                cols {0,33} <- {32,1}, then rows {0,33} <- {32,1} (covers
                corners).  Packing both sides of the wrap into one op via a
                2-count outer dim halves the op count and dep hops."""
                eng = eng or nc.vector
                pitch = pad.ap[0][0]
                co = pad[0:rows, 34:35].copy()
                co.ap = _bass_rust.VecI64Pair([[pitch, rows], [34, 32], [33, 2]])
                ci = pad[0:rows, 66:67].copy()
                ci.ap = _bass_rust.VecI64Pair([[pitch, rows], [34, 32], [-31, 2]])
                eng.tensor_copy(co, ci)
                ro = pad[0:rows, 0:1].copy()
                ro.ap = _bass_rust.VecI64Pair([[pitch, rows], [1122, 2], [1, 34]])
                ri = pad[0:rows, 1088:1089].copy()
                ri.ap = _bass_rust.VecI64Pair([[pitch, rows], [-1054, 2], [1, 34]])
                eng.tensor_copy(ro, ri)

            # ---------------------------------------------------------------
            def vh(ap):  # [p, 512] -> [p, 16, 32]
                return ap.rearrange("p (h w) -> p h w", h=16)

            def build_pad(pad, psA, psB, func, bias, rows, to128):
                """Pad interior <- func(psum_half + bias) in two ACT chunks
                (chunk A only needs the h0=0 conv half, so it overlaps the
                h0=16 matmuls).  With to128, the (x+1)-shifted second plane
                (partitions 64..128, for the kx-pair matmuls) is written by
                two more ACT chunks straight from the same psums, removing
                the serial shifted-copy; its row halos ride the (widened)
                row-halo copies, and only cols 0..32 exist there (the pair
                matmuls read cols 0..32 only)."""
                pv = v34(pad[:, 0:NPAD])
                b = bias if bias is not None else 0.0
                nc.scalar.activation(
                    pv[0:rows, 1:17, 1:33], vh(psA[0:rows, :]), func, bias=b
                )
                nc.scalar.activation(
                    pv[0:rows, 17:33, 1:33], vh(psB[0:rows, :]), func, bias=b
                )
                halos(pad, rows)
                if to128:
                    nc.vector.tensor_copy(
                        pad[64:128, 0 : NPAD - 1], pad[0:64, 1:NPAD]
                    )

            def conv_pair(pairs, sings, pad128, M):
                """3x3 circular conv via kx-paired matmuls into two
                [128, 512] half psums (pixel rows 0..16 / 16..32)."""
                pv = v34(pad128[:, 0:NPAD])
                halves = []
                for h0, tag in ((0, "convA"), (16, "convB")):
                    ps = cpool.tile([128, 512], F32, tag=tag, name=tag)
                    for ky in range(3):
                        nc.tensor.matmul(
                            ps[0:M, 0:512],
                            pairs[:, ky, 0:M],
                            pv[0:128, ky + h0 : ky + h0 + 16, 0:32],
                            start=(ky == 0),
                            stop=False,
                        )
                        nc.tensor.matmul(
                            ps[0:M, 0:512],
                            sings[:, ky, 0:M],
                            pv[0:64, ky + h0 : ky + h0 + 16, 2:34],
                            start=False,
                            stop=(ky == 2),
                        )
                    halves.append(ps)
                return halves

            # ---------------------------------------------------------------
            def encode(src_row_ap, startup=False):
                """src_row_ap: [1, PPAD] fp16 padded image -> upad2 tile."""
                ip9 = pads1.tile([9, NPAD], F16, tag="ip9", name="ip9")
                # 3 gather-DMAs (one per ky) build the 9 shifted windows:
                # dims = [partition x1, kx-window x3 (stride 1, overlapping),
                # row x NPAD] -- a legal 3-dim DMA AP on real hardware.
                # During dec steps avoid the Pool queue: a waiting DMA at
                # the Pool queue head stalls the whole state grind.
                pitch = src_row_ap.ap[0][0]
                qs = (nc.sync, nc.gpsimd, nc.scalar) if startup else (
                    nc.sync, nc.scalar, nc.sync)
                dmas = []
                for ky in range(3):
                    w9 = src_row_ap[0:1, PW * ky : PW * ky + NPAD].copy()
                    w9.ap = _bass_rust.VecI64Pair(
                        [[pitch, 1], [1, 3], [1, NPAD]]
                    )
                    dmas.append(qs[ky].dma_start(ip9[3 * ky : 3 * ky + 3, :], w9))
                if not startup:
                    # re-ramp PE during the ip9 DMA window so enc1/enc2/combo
                    # matmuls start at full clock
                    from concourse.bass import _add_dep_helper as _adh
                    wt9 = cpool.tile([128, 512], F32, tag="convA", name="warm9")
                    for _ in range(3):
                        mm = nc.tensor.matmul(
                            wt9[0:1, 0:64], Msel[:, 0:1], Msel[:, 0:64],
                            start=True, stop=True,
                        )
                        _adh(mm.ins, dmas[0].ins, True, "pe warm enc")
                ip9v = v34(ip9[:])
                p1 = []
                for h0, tag in ((0, "convA"), (16, "convB")):
                    ps1 = cpool.tile([128, 512], F32, tag=tag, name=tag)
                    nc.tensor.matmul(
                        ps1[0:64, 0:512],
                        enc1w[:],
                        ip9v[0:9, h0 : h0 + 16, 0:32],
                        start=True,
                        stop=True,
                    )
                    p1.append(ps1)
                e1 = pads1.tile([128, NPAD], F16, tag="e1pad", name="e1pad")
                build_pad(e1, p1[0], p1[1], AF.Relu, eb1, 64, True)
                p2 = conv_pair(enc2p, enc2s, e1, 64)
                up = pads2.tile([128, NPAD], F16, tag="upad", name="upad")
                build_pad(up, p2[0], p2[1], AF.Relu, eb2, 64, True)
                return up

            # ---------------------------------------------------------------
            def prologue(up, is_dec, tail=False):
                # tail=True: ACT is the serial chain (decoder path) -> put
                # PSUM->SBUF copies on DVE.  tail=False: DVE/Pool grind the
                # state -> keep copies on ACT.
                cp = nc.vector.tensor_copy if tail else nc.scalar.copy
                psc = conv_pair(combop, combos, up, 80)
                # softplus via exp+ln, pixel-halved so the chain pipelines
                # with the conv's second half and the drp/abar halves
                d16 = []
                for i in (0, 1):
                    d = small.tile([8, 512], F16, tag=f"delta{i}", name=f"delta{i}")
                    nc.scalar.activation(d[:], psc[i][0:8, :], AF.Exp, bias=dbias)
                    nc.scalar.activation(d[:], d[:], AF.Ln, bias=1.0)
                    d16.append(d)
                Bv16 = small.tile([16, NP], F16, tag="bv16", name="bv16")
                cp(Bv16[0:16, 0:512], psc[0][32:48, :])
                cp(Bv16[0:16, 512:1024], psc[1][32:48, :])
                Cv16 = None
                cpc = nc.scalar.copy if tail else cp
                if is_dec:
                    Cv16 = small.tile([16, NP], F16, tag="cv16", name="cv16")
                    cpc(Cv16[0:16, 0:512], psc[0][64:80, :])
                    cpc(Cv16[0:16, 512:1024], psc[1][64:80, :])

                abar = probuf.tile([128, NP], F16, tag="abar", name="abar")
                drp = []
                for i, tag in ((0, "convA"), (1, "convB")):
                    dr = cpool.tile([128, 512], F32, tag=tag, name=tag)
                    nc.tensor.matmul(
                        dr[:, 0:512], Ed[:], d16[i][:, 0:512],
                        start=True, stop=True,
                    )
                    nc.scalar.activation(
                        abar[:, 512 * i : 512 * i + 512], dr[:], AF.Exp,
                        scale=App,
                    )
                    drp.append(dr)

                upv = v34(up[:, 0:NPAD])
                urs = []
                for i, tag in ((0, "convA"), (1, "convB")):
                    urp = cpool.tile([128, 512], F32, tag=tag, name=tag)
                    nc.tensor.matmul(
                        urp[:, 0:512],
                        Gu[:],
                        upv[0:64, 1 + 16 * i : 17 + 16 * i, 1:33],
                        start=True,
                        stop=True,
                    )
                    ur = probuf.tile([128, 512], F32, tag=f"ur{i}", name=f"ur{i}")
                    cp(ur[:], urp[:])
                    urs.append(ur)

                bt = bpool.tile([128, NP], F16, tag="b", name="bt")
                for i, tag in ((0, "convA"), (1, "convB")):
                    bvp = cpool.tile([128, 512], F32, tag=tag, name=tag)
                    nc.tensor.matmul(
                        bvp[:, 0:512], Fs[:], Bv16[:, 512 * i : 512 * i + 512],
                        start=True, stop=True,
                    )
                    # ur <- (Bv_rep * invA) * u_rep
                    nc.vector.scalar_tensor_tensor(
                        out=urs[i][:], in0=bvp[:], scalar=invApp, in1=urs[i][:],
                        op0=OP.mult, op1=OP.mult,
                    )
                    nc.vector.scalar_tensor_tensor(
                        out=bt[:, 512 * i : 512 * i + 512],
                        in0=abar[:, 512 * i : 512 * i + 512],
                        scalar=-1.0, in1=urs[i][:],
                        op0=OP.add, op1=OP.mult,
                    )
                cvr = None
                if is_dec:
                    cvr = probuf.tile([128, NP], F16, tag="cvrep", name="cvrep")
                    for i, tag in ((0, "convA"), (1, "convB")):
                        cvp = cpool.tile([128, 512], F32, tag=tag, name=tag)
                        nc.tensor.matmul(
                            cvp[:, 0:512], Fs[:],
                            Cv16[:, 512 * i : 512 * i + 512],
                            start=True, stop=True,
                        )
                        cpc(cvr[:, 512 * i : 512 * i + 512], cvp[:])
                return {"abar": abar, "b": bt, "cvr": cvr, "up": up}

            # ---------------------------------------------------------------
            def emit_y(v, w, first, acc, eng, ek):
                """8 matmuls into an engine-private [128, 256] PSUM ring; one
                max per velocity into the engine-local accumulator.  For DVE
                velocities the idle ACT engine stages PSUM->SBUF fp16 so the
                DVE max runs in 2x fp16 mode."""
                pool = ypool if ek == "D" else gpool
                yp = pool.tile([128, 256], F32, tag=f"yp{ek}", name="yp")
                # dep-free filler into this velocity's own psum (overwritten
                # by the start=True matmul below): keeps the PE queue
                # non-empty between velocity groups so the p-state ramp
                # holds and the Msel matmuls run at 0.417 ns/row
                nc.tensor.matmul(
                    yp[0:1, 0:64], Msel[:, 0:1], Msel[:, 0:64],
                    start=True, stop=True,
                )
                for half in range(2):
                    for jj in range(4):
                        j = 4 * half + jj
                        nc.tensor.matmul(
                            yp[:, 128 * half : 128 * half + 128],
                            Msel[:, 128 * j : 128 * j + 128],
                            w[:, 128 * j : 128 * j + 128],
                            start=(jj == 0),
                            stop=(jj == 3),
                        )
                # GPSIMD can neither read PSUM nor run max on real HW: ACT
                # stages PSUM->SBUF fp16, DVE owns every max (2x fp16 mode).
                if first:
                    nc.scalar.copy(acc[:], yp[:])
                    return
                ysb = wbuf.tile([128, 256], F16, tag=f"ysb{ek}", name="ysb")
                nc.scalar.copy(ysb[:], yp[:])
                nc.vector.tensor_tensor(
                    out=acc[:], in0=ysb[:], in1=acc[:], op=OP.max
                )

            def grind_y(pro, ymax, after_g=None, after_slab=None):
                """y-path: w = (abar*cvr)*s_rot per velocity, running max,
                then += Msel(cvr*b) after the max.  Reads only the OLD state,
                so it runs before the update and the AllGather launches
                early."""
                acv = ybuf.tile([128, NP], F16, tag="acv", name="acv")
                nc.vector.tensor_tensor(
                    out=acv[:], in0=pro["abar"][:], in1=pro["cvr"][:],
                    op=OP.mult,
                )
                cb16 = wbuf.tile([128, NP], F16, tag="cb16", name="cb16")
                cbi = nc.gpsimd.tensor_tensor(
                    out=cb16[:], in0=pro["cvr"][:], in1=pro["b"][:],
                    op=OP.mult,
                )
                from concourse.bass import _add_dep_helper as _adh
                if after_g is not None:
                    # same-engine ordering edge: the previous cell's Pool
                    # update slice must precede this grind's Pool work, or
                    # the scheduler (whose pass mocks the collective cost)
                    # pushes the update into the grind window
                    _adh(cbi.ins, after_g.ins, True, "pool upd before y")
                acvv = v3(acv[:])
                acvg = acv
                if after_slab is not None:
                    # bypass-op barrier: bit-copies acv while READING the
                    # last slab written by the previous cell's Pool update
                    # slice.  Pool y-mults read this copy, so the scheduler
                    # cannot queue them (which wait on acv, ~28us away)
                    # ahead of the ready update work.
                    acvg = ybuf.tile([128, NP], F16, tag="acv2", name="acv2")
                    nc.gpsimd.tensor_tensor(
                        out=acvg[:], in0=acv[:], in1=after_slab[:],
                        op=OP.bypass,
                    )
                acvgv = v3(acvg[:])
                ymax2 = ybuf.tile([128, 256], F16, tag="ymaxB", name="ymaxB")
                firsts = {0: True, 1: True}
                # v=12 (zero shift, 1 rect, DVE) goes last so the final
                # mult->matmul->max tail before the collective is minimal
                vorder = [v for v in range(NV) if v != 12] + [12]
                for vi, v in enumerate(vorder):
                    vx, vy = V_LIST[v]
                    ek = "G" if v in GP_VS_Y else "D"
                    eng = nc.gpsimd if ek == "G" else nc.vector
                    w = wbuf.tile([128, NP], F16, tag="w16", name="w16y")
                    wv = v3(w[:])
                    srcv = v3(slabs[vtile[v]][:])
                    av = acvgv if ek == "G" else acvv
                    for oh, ih, hl, ow, iw, wl in _rects(vy, vx):
                        bi = eng.tensor_tensor(
                            out=wv[:, oh : oh + hl, ow : ow + wl],
                            in0=srcv[:, ih : ih + hl, iw : iw + wl],
                            in1=av[:, oh : oh + hl, ow : ow + wl],
                            op=OP.mult,
                        )
                        if ek == "G" and after_g is not None:
                            _adh(bi.ins, after_g.ins, True, "pool upd first")
                    par = v & 1
                    emit_y(v, w, firsts[par], ymax if par == 0 else ymax2,
                           eng, ek)
                    firsts[par] = False
                nc.vector.tensor_tensor(
                    out=ymax[:], in0=ymax2[:], in1=ymax[:], op=OP.max
                )
                # += sum_s Cv*b  (same for all v, add after max)
                ybp = ypool.tile([128, 256], F32, tag="ypD", name="ybp")
                for half in range(2):
                    for jj in range(4):
                        j = 4 * half + jj
                        nc.tensor.matmul(
                            ybp[:, 128 * half : 128 * half + 128],
                            Msel[:, 128 * j : 128 * j + 128],
                            cb16[:, 128 * j : 128 * j + 128],
                            start=(jj == 0),
                            stop=(jj == 3),
                        )
                nc.vector.tensor_tensor(
                    out=ymax[:], in0=ybp[:], in1=ymax[:], op=OP.add
                )

            def grind_update(cell, pro, dec=False, only=None):
                gset = GP_VS_UPD_DEC if dec else GP_VS_UPD
                src_b1 = pro0_b if cell == 2 else None
                abv = v3(pro["abar"][:])
                last_g = None
                last_gsp = None
                for v in range(NV):
                    if only is not None and v not in only:
                        continue
                    vx, vy = V_LIST[v]
                    eng = nc.gpsimd if v in gset else nc.vector
                    ek = "G" if v in gset else "D"
                    sp = slabs[spare[ek]]
                    spv = v3(sp[:])
                    src = src_b1 if src_b1 is not None else slabs[vtile[v]]
                    srcv = v3(src[:])
                    for oh, ih, hl, ow, iw, wl in _rects(vy, vx):
                        bi = eng.tensor_tensor(
                            out=spv[:, oh : oh + hl, ow : ow + wl],
                            in0=srcv[:, ih : ih + hl, iw : iw + wl],
                            in1=abv[:, oh : oh + hl, ow : ow + wl],
                            op=OP.mult,
                        )
                    bi = eng.tensor_tensor(
                        out=sp[:], in0=sp[:], in1=pro["b"][:], op=OP.add
                    )
                    if ek == "G":
                        last_g = bi
                        last_gsp = sp
                    # rotate spare within engine group
                    spare[ek], vtile[v] = vtile[v], spare[ek]
                return last_g, last_gsp

            # ---------------------------------------------------------------
            def post_y_launch(ymax, pro, warm_pe=True):
                """AllGather launch, emitted BEFORE the state update so the
                collective runs concurrently with it.  The ymax->yf layout
                shuffle rides the staging DMAs (4 gathers with a strided
                DRAM-side AP) instead of costing DVE copies after the last
                max: cc_in[d, 512h+128q+n] = ymax[32q+d, 128h+n]."""
                cc_in = dram.tile([8, NP], F16, tag="ccin", name="ccin")
                cc_out = dram.tile([64, NP], F16, tag="ccout", name="ccout")
                qeng = (nc.sync, nc.scalar, nc.gpsimd, nc.sync)
                for q in range(4):
                    o = cc_in[0:8, 128 * q : 128 * q + 128].copy()
                    o.ap = _bass_rust.VecI64Pair([[NP, 8], [512, 2], [1, 128]])
                    qeng[q].dma_start(o, ymax[32 * q : 32 * q + 8, 0:256])
                # Issue the collective from the PE queue: PE's next queued
                # work (decoder matmuls) depends on the gathered result
                # anyway, so the 18us cost-model charge stalls nothing,
                # whereas on the Pool queue it blocked the state grind.
                from concourse.bass import BassGpSimd, _add_dep_helper

                cc = BassGpSimd.collective_compute(
                    {"GP": nc.gpsimd, "ACT": nc.scalar, "DVE": nc.vector, "PE": nc.tensor}[os.environ.get("CC_ENG", "GP")],
                    "AllGather",
                    OP.bypass,
                    replica_groups=[list(range(NCORES))],
                    ins=[cc_in.opt()],
                    outs=[cc_out.opt()],
                )
                # PE warmers: the p-state resets during the collective+DMA
                # gap; a short burst of scratch matmuls pinned (via explicit
                # deps) to run right after the collective re-ramps PE so the
                # decoder convs start at full clock instead of LOW/MID.
                def warm(n, dep):
                    wt = cpool.tile([128, 512], F32, tag="convA", name="warmcc")
                    for _ in range(n):
                        mm = nc.tensor.matmul(
                            wt[0:1, 0:64], Msel[:, 0:1], Msel[:, 0:64],
                            start=True, stop=True,
                        )
                        _add_dep_helper(mm.ins, dep.ins, True, "pe warm")

                warm(4, cc)
                ytmp = ybuf.tile([64, NP], F16, tag="ytmp", name="ytmp")
                dma1 = nc.sync.dma_start(ytmp[0:64, 0:512], cc_out[0:64, 0:512])
                nc.scalar.dma_start(ytmp[0:64, 512:1024], cc_out[0:64, 512:1024])
                warm(3, dma1)
                # du = Dskip*u on the idle ACT engine, so the post-collective
                # ypad build is a 2x-mode fp16 add instead of a full-rate STT
                du = ybuf.tile([64, NP], F16, tag="du", name="du")
                nc.scalar.activation(
                    du[:], v34(pro["up"][:, 0:NPAD])[0:64, 1:33, 1:33],
                    AF.Identity, scale=dsk,
                )
                return ytmp, du

            def post_y(step, pro, ytmp, du, last):
                # ypad interior = du + y  (du = Dskip*u precomputed on ACT)
                yp2 = pads1.tile([128, NPAD], F16, tag="ypad", name="ypad")
                ypv = v34(yp2[:, 0:NPAD])
                nc.vector.tensor_tensor(
                    out=ypv[0:64, 1:33, 1:33],
                    in0=v3(du[:]),
                    in1=v3(ytmp[:]),
                    op=OP.add,
                )
                halos(yp2, 64)
                nc.vector.tensor_copy(yp2[64:128, 0 : NPAD - 1], yp2[0:64, 1:NPAD])

                pd1 = conv_pair(dec1p, dec1s, yp2, 64)
                d1 = pads1.tile([128, NPAD], F16, tag="d1pad", name="d1pad")
                build_pad(d1, pd1[0], pd1[1], AF.Relu, db1, 64, True)
                pd2 = conv_pair(dec2p, dec2s, d1, 64)
                d2 = pads1.tile([128, NPAD], F16, tag="d2pad", name="d2pad")
                build_pad(d2, pd2[0], pd2[1], AF.Relu, db2, 64, True)
                pd3 = conv_pair(dec3p, dec3s, d2, 1)
                if not last:
                    # dec3's ACT writes the prediction directly into the
                    # padded layout (strided out AP), skipping the separate
                    # pred16 tile + DVE interior copy on the serial chain
                    pp = pads1.tile([1, PPAD], F16, tag="predpad", name="predpad")
                    if step < 2:
                        # predpad tag has its own 2-slot ring; the NPAD:PPAD
                        # slack is never overwritten, so only the first use
                        # of each slot needs the zero-fill
                        nc.vector.memset(pp[0:1, NPAD:PPAD], 0.0)
                    ppv = v34(pp[:, 0:NPAD])
                    nc.scalar.activation(
                        ppv[0:1, 1:17, 1:33], vh(pd3[0][0:1, :]),
                        AF.Identity, bias=db3,
                    )
                    nc.scalar.activation(
                        ppv[0:1, 17:33, 1:33], vh(pd3[1][0:1, :]),
                        AF.Identity, bias=db3,
                    )
                    halos(pp, 1)
                    # d_out reads the padded interior (Pool queue idles
                    # through this stretch and the wait clears immediately)
                    nc.gpsimd.dma_start(
                        d_out[step : step + 1, :].rearrange(
                            "p (h w) -> p h w", h=H
                        ),
                        ppv[0:1, 1:33, 1:33],
                    )
                    return pp
                pred16 = ybuf.tile([1, NP], F16, tag="pred16", name="pred16")
                nc.scalar.activation(
                    pred16[0:1, 0:512], pd3[0][0:1, :], AF.Identity, bias=db3
                )
                nc.scalar.activation(
                    pred16[0:1, 512:1024], pd3[1][0:1, :], AF.Identity,
                    bias=db3,
                )
                nc.scalar.dma_start(d_out[step : step + 1, :], pred16[:])
                return None

            # ================= main sequence =================
            pros = []
            for t in range(t_in):
                up = encode(d_ipad[t : t + 1, :], startup=True)
                pros.append(prologue(up, is_dec=(t == t_in - 1)))

            # decoder weights load after the startup-critical DMAs
            dec1p = load(d_dec1p, [128, 3, DM], F16, "dec1p")
            dec1s = load(d_dec1s, [64, 3, DM], F16, "dec1s")
            dec2p = load(d_dec2p, [128, 3, DM], F16, "dec2p")
            dec2s = load(d_dec2s, [64, 3, DM], F16, "dec2s")
            dec3p = load(d_dec3p, [128, 3, 1], F16, "dec3p")
            dec3s = load(d_dec3s, [64, 3, 1], F16, "dec3s")
            pro0_b = pros[0]["b"]

            # enc cells 2..t_in  (cell 1 is implicit: s_1 = b_1).
            # Moderate priority boost: let the scheduler start these D/G
            # ops as soon as their prologue is ready instead of after the
            # whole encode pipeline drains.
            for cell in range(2, t_in + 1):
                grind_update(cell, pros[cell - 1])  # tuple return unused

            # dec steps.  The collective must issue from the Pool queue
            # (only gpsimd collectives survive the HW lowering) and the
            # queue is in-order, so Pool's update share is split: a small
            # pre-launch slice (so the collective isn't delayed), with the
            # rest emitted after the launch so it executes during the
            # 18us collective + decoder chain instead of stalling behind it.
            upd_pool = sorted(GP_VS_UPD_DEC)
            upd_pre = set(upd_pool[:0])
            upd_post = set(upd_pool[0:])
            upd_dve = set(range(NV)) - set(upd_pool)
            prev_g = None
            prev_slab = None
            for k in range(1, plen + 1):
                cell = t_in + k
                pro = pros[t_in - 1 + (k - 1)]
                ymax = ybuf.tile([128, 256], F16, tag="ymaxA", name="ymaxA")
                grind_y(pro, ymax, after_g=prev_g, after_slab=None)
                if cell < n_cells:
                    grind_update(cell, pro, dec=True, only=upd_pre)
                ytmp, du = post_y_launch(ymax, pro, warm_pe=(k < plen))
                prev_g, prev_slab = None, None
                if cell < n_cells:
                    prev_g, prev_slab = grind_update(cell, pro, dec=True, only=upd_post)
                    grind_update(cell, pro, dec=True, only=upd_dve)
                pp = post_y(k - 1, pro, ytmp, du, last=(k == plen))
                if k < plen:
                    up = encode(pp[0:1, :])
                    pros.append(prologue(up, is_dec=True, tail=True))

    _split_waits(nc)
    return nc


# ---------------------------------------------------------------------------
def _pad_img(x):
    """[32,32] -> [PPAD] fp16 padded-wrap flat."""
    p = np.pad(x, 1, mode="wrap")
    out = np.zeros(PPAD, np.float16)
    out[:NPAD] = p.reshape(-1).astype(np.float16)
    return out


def _pack_pair(w):
    """conv weight [M, cin, 3, 3] -> pair [128,3,M], single [64,3,M] fp16."""
    M, cin = w.shape[0], w.shape[1]
    pair = np.zeros((128, 3, M), np.float16)
    sing = np.zeros((64, 3, M), np.float16)
    for ky in range(3):
        pair[:cin, ky, :] = w[:, :, ky, 0].T
        pair[64 : 64 + cin, ky, :] = w[:, :, ky, 1].T
        sing[:cin, ky, :] = w[:, :, ky, 2].T
    return pair, sing


_CACHE = {}


def kernel(**inputs):
    input_seq = np.asarray(inputs["input_seq"], np.float32)
    B, t_in, C, Hh, Ww = input_seq.shape
    assert B == 1 and C == 1 and Hh == H and Ww == H
    plen = int(np.asarray(inputs["pred_len"]))

    key = (t_in, plen)
    if key not in _CACHE:
        _CACHE[key] = build_program(t_in, plen)
    nc = _CACHE[key]

    w1 = np.asarray(inputs["enc_w1"], np.float32)
    enc1w = np.zeros((9, DM), np.float16)
    for ky in range(3):
        for kx in range(3):
            enc1w[3 * ky + kx, :] = w1[:, 0, ky, kx]
    enc2p, enc2s = _pack_pair(np.asarray(inputs["enc_w2"], np.float32))
    dec1p, dec1s = _pack_pair(np.asarray(inputs["dec_w1"], np.float32))
    dec2p, dec2s = _pack_pair(np.asarray(inputs["dec_w2"], np.float32))
    dec3p, dec3s = _pack_pair(np.asarray(inputs["dec_w3"], np.float32))

    ipad = np.stack([_pad_img(input_seq[0, t, 0]) for t in range(t_in)])

    Ed = np.zeros((8, 128), np.float16)
    for d in range(8):
        Ed[d, d * 16 : (d + 1) * 16] = 1
    Fs = np.zeros((16, 128), np.float16)
    for s in range(16):
        Fs[s, s::16] = 1
    Msel = np.zeros((128, 8, 128), np.float16)
    for j in range(8):
        for d in range(8):
            Msel[d * 16 : (d + 1) * 16, j, 32 * (j % 4) + d] = 1
    Msel = Msel.reshape(128, 1024)

    logA = np.asarray(inputs["log_A_real"], np.float32)
    wd = np.asarray(inputs["wd"], np.float32)
    wB = np.asarray(inputs["wB"], np.float32)
    wC = np.asarray(inputs["wC"], np.float32)
    bd = np.asarray(inputs["bd"], np.float32)
    dt_inv = float(np.asarray(inputs["dt_inv"]))

    def col(x):
        return np.ascontiguousarray(x.reshape(-1, 1), np.float32)

    shared = {
        "ipad": ipad,
        "enc1w": enc1w,
        "enc2p": enc2p, "enc2s": enc2s,
        "dec1p": dec1p, "dec1s": dec1s,
        "dec2p": dec2p, "dec2s": dec2s,
        "dec3p": dec3p, "dec3s": dec3s,
        "Ed": Ed, "Fs": Fs, "Msel": Msel,
    }

    in_maps = []
    for c in range(NCORES):
        sl = slice(8 * c, 8 * c + 8)
        wcombo = np.zeros((80, 64, 3, 3), np.float32)
        wcombo[0:8] = wd[sl]
        wcombo[32:48] = wB
        wcombo[64:80] = wC
        cp, cs = _pack_pair(wcombo)
        A = -np.exp(logA[sl])  # [8, 16]
        Gu = np.zeros((64, 128), np.float16)
        for d in range(8):
            Gu[8 * c + d, d * 16 : (d + 1) * 16] = 1
        biasp = np.zeros((128, 9), np.float32)
        biasp[:, 0] = A.reshape(-1)
        biasp[:, 1] = (1.0 / A).reshape(-1)
        biasp[0:8, 2] = bd[sl] + dt_inv
        biasp[0:64, 3] = np.asarray(inputs["enc_b1"], np.float32)
        biasp[0:64, 4] = np.asarray(inputs["enc_b2"], np.float32)
        biasp[0:64, 5] = np.asarray(inputs["dec_b1"], np.float32)
        biasp[0:64, 6] = np.asarray(inputs["dec_b2"], np.float32)
        biasp[0:64, 7] = np.asarray(inputs["Dskip"], np.float32)
        biasp[0:1, 8] = np.asarray(inputs["dec_b3"], np.float32).reshape(-1)[0]
        m = dict(shared)
        m.update(
            {
                "combop": cp,
                "combos": cs,
                "Gu": Gu,
                "biasp": biasp,
            }
        )
        in_maps.append(m)

    res = run_bass_kernel_spmd(nc, in_maps, list(range(NCORES)))
    preds = res.results[0]["preds"]  # [plen, 1024]
    return preds.reshape(1, plen, 1, H, H).astype(np.float32)


if __name__ == "__main__":
    # lightweight shape self-check without hardware: just build the program
    nc = build_program(4, 4)
    n = sum(len(bb.instructions) for f in nc.m.functions for bb in f.blocks)
    print("program built, instructions:", n)

